# revision 2
# baseline (speedup 1.0000x reference)
"""Trainium2 Bass kernel for MultiHeadSelfAttention (K-only variant).

Math (per batch b):
    K  = x @ Wk.T;  Kh = heads(K)
    S_h = Kh @ Kh.T / sqrt(D);  P_h = softmax(S_h)
    wV_h = P_h @ Kh  (V == K);  out = concat_h(wV) @ Wo.T

Sharding (8 cores): core c handles batch c//2 and query-half c%2 with all
heads.  The query half is selected by rolling x on the host so each core
always computes queries 0:S//2 of its (rolled) sequence; softmax over keys
is order-invariant so rolling the key axis is harmless.

Per-core pipeline (one SPMD NEFF):
    xT_bf  = XBAR-DMA-transpose(bf16(x))     [d, s]
    K      = xT_bf.T @ WkT_bf  (bf16 matmuls, fp32 psum) -> k_bf, kones
    khT    = XBAR-DMA-transpose(k_bf)        [e, s] bf16
    per (qb, head):
      scores strip S_T[k, q] = khT_h.T @ khT_h[:, qb]    (bf16)
      E_T = exp(S_T / sqrt(D))       (ScalarE, psum -> sbuf bf16)
      PV:  [wVT_h ; rowsum_h] = [Kh_h | 1].T @ E_T       (bf16, psum accum)
      recip = 1/rowsum; partition-broadcast via K=1 matmul; normalize wVT
    out = wVTn.T @ WoT  (fp32r), first half overlapped with second qb
"""

import sys

if "/opt/trn_rl_repo" not in sys.path:
    sys.path.insert(0, "/opt/trn_rl_repo")

import numpy as np

B, S, D = 4, 2048, 512
H = 8
HD = D // H            # 64
P = 128
SH = S // 2            # query half per core = 1024
NCORES = 8
SCALE = 1.0 / np.sqrt(D)

_CACHE = {}


def _build_nc(repeat: int = 1, mode: str = "full"):
    import concourse.bass as bass  # noqa: F401
    import concourse.tile as tile
    import concourse.mybir as mybir
    from concourse import bacc
    from concourse.masks import make_identity
    from contextlib import ExitStack

    f32 = mybir.dt.float32
    f32r = mybir.dt.float32r
    bf16 = mybir.dt.bfloat16

    nc = bacc.Bacc("TRN2", target_bir_lowering=False, debug=False,
                   num_devices=NCORES)

    x_d = nc.dram_tensor("x", [S, D], f32, kind="ExternalInput").ap()
    wk_d = nc.dram_tensor("Wk", [D, D], f32, kind="ExternalInput").ap()
    wo_d = nc.dram_tensor("Wo", [D, D], f32, kind="ExternalInput").ap()
    out_d = nc.dram_tensor("out", [SH, D], f32, kind="ExternalOutput").ap()

    NSC = S // P           # 16 sequence chunks
    NDC = D // P           # 4 feature chunks
    NQB = SH // 512        # 2 query blocks of 512
    QB = 512

    import contextlib
    with tile.TileContext(nc) as tc:
        loop_cm = tc.For_i(0, repeat, 1) if repeat > 1 else contextlib.nullcontext()
        with loop_cm, ExitStack() as ctx:
            consts = ctx.enter_context(tc.tile_pool(name="consts", bufs=1))
            kpool = ctx.enter_context(tc.tile_pool(name="kpool", bufs=1))
            _ebufs = 5 if "ct" in mode else (3 if "e3" in mode else 4)
            epool = ctx.enter_context(
                tc.tile_pool(name="epool", bufs=_ebufs))
            vpool = ctx.enter_context(tc.tile_pool(name="vpool", bufs=1))
            opool = ctx.enter_context(
                tc.tile_pool(name="opool", bufs=(3 if "x4" in mode else 2)))
            # psum: tag A = 4 banks x1, tag B = 2 banks x2  -> 8 banks total
            ps = ctx.enter_context(tc.tile_pool(name="ps", bufs=1, space="PSUM"))

            bigspan = "bigspan" in mode

            def spanA(dtype=None):
                return ps.tile([P, 4, 512], dtype or f32, tag="A",
                               bufs=(2 if bigspan else 1), name="spA")

            def spanB(shape=None, name="spB"):
                if bigspan:
                    return ps.tile(shape or [P, 2, 512], f32, tag="A", bufs=2,
                                   name=name)
                return ps.tile(shape or [P, 2, 512], f32, tag="B", bufs=2,
                               name=name)

            ident = consts.tile([P, P], f32)
            make_identity(nc, ident[:])

            ones1x64f = consts.tile([1, 64], f32)
            nc.gpsimd.memset(ones1x64f[:], 1.0)
            ones1x64 = consts.tile([1, 64], f32r)
            nc.vector.tensor_copy(ones1x64[:], ones1x64f[:])
            ones_bf = consts.tile([P, 1], bf16)
            nc.gpsimd.memset(ones_bf[:], 1.0)
            ident_bf = consts.tile([P, P], bf16)
            nc.vector.tensor_copy(ident_bf[:], ident[:])

            woT = consts.tile([P, NDC, 512], f32r)      # [d', e]
            k_bf = kpool.tile([P, NSC, 512], bf16)      # K [s, e]
            kones = (None if "ct" in mode else
                     kpool.tile([P, NSC, H, HD + 1], bf16))
            khT = kpool.tile([P, NDC, S], bf16)         # K^T [e, s]
            wvt = vpool.tile([P, NDC, SH], f32r)        # wVT (norm in place)

            # ---- phase 0/1: weights, x -> xT (PE transposes) -> K proj -----
            with tc.tile_pool(name="stage", bufs=1) as stage:
                wk_sb = stage.tile([P, NDC, 512], f32, tag="w")
                nc.sync.dma_start(wk_sb[:], wk_d.rearrange("(eo p) d -> p eo d", p=P))
                wkT_r = consts.tile([P, NDC, 512], f32r)
                sp = spanA()
                for dc in range(NDC):
                    for eo in range(NDC):
                        nc.tensor.transpose(
                            sp[:, dc, eo * P:(eo + 1) * P],
                            wk_sb[:, eo, dc * P:(dc + 1) * P], ident[:])
                nc.scalar.copy(wkT_r[:], sp[:])

                # x: 8 groups of 2 seq-chunks; PE transpose -> fp32r Kproj
                for g in range(8):
                    g0 = g * 2
                    x_g = stage.tile([P, 2, 512], f32, tag="x",
                                     bufs=(4 if "x4" in mode else 3),
                                     name="x_g")
                    nc.sync.dma_start(
                        x_g[:, 0:2, :],
                        x_d[g0 * P:(g0 + 2) * P, :].rearrange(
                            "(two p) d -> p two d", p=P))
                    spt = spanB([P, 4, 256], name="sptr")
                    for i in range(2):
                        for dc in range(NDC):
                            nc.tensor.transpose(
                                spt[:, dc, i * P:(i + 1) * P],
                                x_g[:, i, dc * P:(dc + 1) * P], ident[:])
                    xT_g = stage.tile([P, NDC, 256], f32r, tag="xT", bufs=2,
                                      name="xT_g")
                    nc.scalar.copy(xT_g[:], spt[:])

                    spk = spanB(name="spkp")
                    for i in range(2):
                        for dc in range(NDC):
                            nc.tensor.matmul(
                                spk[:, i, :],
                                xT_g[:, dc, i * P:(i + 1) * P],
                                wkT_r[:, dc, :],
                                start=(dc == 0), stop=(dc == NDC - 1))
                    nc.vector.tensor_copy(k_bf[:, g0:g0 + 2, :], spk[:, 0:2, :])
                    if kones is not None:
                        nc.vector.tensor_copy(
                            kones[:, g0:g0 + 2, :, 0:HD],
                            spk[:, 0:2, :].rearrange("p g (h e) -> p g h e",
                                                     h=H))
                    if "pekt" in mode:
                        # khT via PE transposes (bf16): 8 tiles -> one A span
                        spkt = spanA(bf16)
                        for i in range(2):
                            sc = g0 + i
                            for ec in range(NDC):
                                nc.tensor.transpose(
                                    spkt[:, ec, i * P:(i + 1) * P],
                                    k_bf[:, sc, ec * P:(ec + 1) * P],
                                    ident_bf[:])
                        nc.vector.tensor_copy(
                            khT[:, :, g0 * P:(g0 + 2) * P],
                            spkt[:, 0:NDC, 0:2 * P])

                # Wo DMA early (keeps all DMACopies before the XBAR
                # cluster); its PE transposes run after the khT transposes
                # so head-0 scores can start as soon as khT chunks land.
                wo_sb = stage.tile([P, NDC, 512], f32, tag="w")
                nc.sync.dma_start(wo_sb[:], wo_d.rearrange("(eo p) d -> p eo d", p=P))

                if "pekt" not in mode:
                    # khT via XBAR transposes, one cluster after all DMAs
                    for sc in range(NSC):
                        nc.sync.dma_start_transpose(
                            khT[:, :, sc * P:(sc + 1) * P], k_bf[:, sc, :])

                # Wo: transpose via PE (fp32 -> fp32r), span A
                sp = spanA()
                for dc in range(NDC):
                    for eo in range(NDC):
                        nc.tensor.transpose(
                            sp[:, dc, eo * P:(eo + 1) * P],
                            wo_sb[:, eo, dc * P:(dc + 1) * P], ident[:])
                nc.scalar.copy(woT[:], sp[:])

            if kones is not None:
                nc.gpsimd.memset(kones[:, :, :, HD:HD + 1], 1.0)

            if "phase0" in mode:
                # consume everything so DCE cannot strip phase 0/1 work
                with tc.tile_pool(name="sink", bufs=1, space="DRAM") as sink:
                    snk1 = sink.tile([P, NSC, 512], bf16, name="snk1")
                    nc.sync.dma_start(snk1[:], k_bf[:])
                    snk2 = sink.tile([P, NDC, S], bf16, name="snk2")
                    nc.sync.dma_start(snk2[:], khT[:])
                    snk3 = sink.tile([P, NSC, H, HD + 1], bf16, name="snk3")
                    nc.sync.dma_start(snk3[:], kones[:])
                    snk4 = sink.tile([P, NDC, 512], f32, name="snk4")
                    nc.sync.dma_start(snk4[:], woT[:].bitcast(f32))
                o_sb0 = opool.tile([P, 2, 512], f32, tag="osb", name="o_sb0")
                nc.vector.tensor_copy(o_sb0[:, 0, :], khT[:, 0, 0:512])
                nc.vector.tensor_copy(o_sb0[:, 1, :], k_bf[:, 0, :])
                nc.sync.dma_start(
                    out_d[0:2 * P, :].rearrange("(two p) d -> p two d", p=P),
                    o_sb0[:])

            # ---- head loop (qb outer), software-pipelined PV ----------------
            if bigspan:
                kc_groups = [(0, "A"), (4, "A"), (8, "A"), (12, "A")]
            else:
                kc_groups = [(0, "A"), (4, "B"), (6, "B"), (8, "A"),
                             (12, "B"), (14, "B")]

            def emit_pv(h, qb, e_t):
                hp = (h % 2) * HD
                ec = h // 2
                pv = spanB([HD + 1, 512], name="pv")
                for kc in range(NSC):
                    nc.tensor.matmul(
                        pv[:], kones[:, kc, h, :], e_t[:, kc, :],
                        start=(kc == 0), stop=(kc == NSC - 1))
                nc.vector.tensor_copy(
                    wvt[hp:hp + HD, ec, qb * QB:(qb + 1) * QB], pv[0:HD, :])
                recip_t = vpool.tile([1, 512], f32r, tag="recip", bufs=4,
                                     name="recip_t")
                with nc.allow_low_precision(reason="fp32r recip is fine"):
                    nc.vector.reciprocal(recip_t[:], pv[HD:HD + 1, :])
                bc = spanB([HD, 512], name="bc")
                nc.tensor.matmul(
                    bc[:], ones1x64[:], recip_t[:], start=True, stop=True)
                nc.vector.tensor_tensor(
                    wvt[hp:hp + HD, ec, qb * QB:(qb + 1) * QB],
                    wvt[hp:hp + HD, ec, qb * QB:(qb + 1) * QB],
                    bc[:], mybir.AluOpType.mult)

            def emit_outproj(qc0):
                # two q-chunks of 128 per pass, psum in a B slot
                po = spanB(name="po")
                for j in range(2):
                    qc = qc0 + j
                    for dc in range(NDC):
                        nc.tensor.matmul(
                            po[:, j, :],
                            wvt[:, dc, qc * P:(qc + 1) * P],
                            woT[:, dc, :],
                            start=(dc == 0), stop=(dc == NDC - 1))
                o_sb = opool.tile([P, 2, 512], f32, tag="osb", name="o_sb")
                nc.vector.tensor_copy(o_sb[:], po[:])
                nc.sync.dma_start(
                    out_d[qc0 * P:(qc0 + 2) * P, :].rearrange(
                        "(two p) d -> p two d", p=P),
                    o_sb[:])

            def emit_pv_pair(j, qb, e_lo, e_hi):
                # heads (2j, 2j+1) concurrently via PE column tiling
                pv = spanB([P, 512], name="pvp")
                for kc in range(NSC):
                    nc.tensor.matmul(
                        pv[0:HD, :],
                        k_bf[:, kc, (2 * j) * HD:(2 * j + 1) * HD],
                        e_lo[:, kc, :],
                        start=(kc == 0), stop=(kc == NSC - 1),
                        tile_position=(0, 0))
                    nc.tensor.matmul(
                        pv[HD:2 * HD, :],
                        k_bf[:, kc, (2 * j + 1) * HD:(2 * j + 2) * HD],
                        e_hi[:, kc, :],
                        start=(kc == 0), stop=(kc == NSC - 1),
                        tile_position=(0, HD))
                nc.vector.tensor_copy(
                    wvt[:, j, qb * QB:(qb + 1) * QB], pv[:])

            def emit_rs_quad(g, qb, e_ts):
                # rowsums of heads 4g..4g+3 via 4-way column tiling (M=1)
                rs = spanB([97, 512], name="rs")
                for kc in range(NSC):
                    for hi in range(4):
                        nc.tensor.matmul(
                            rs[32 * hi:32 * hi + 1, :],
                            ones_bf[:, 0:1],
                            e_ts[hi][:, kc, :],
                            start=(kc == 0), stop=(kc == NSC - 1),
                            tile_position=(0, 32 * hi))
                for hi in range(4):
                    h = 4 * g + hi
                    hp = (h % 2) * HD
                    ec = h // 2
                    recip_t = vpool.tile([1, 512], f32r, tag="recip", bufs=4,
                                         name="recip_t")
                    with nc.allow_low_precision(reason="fp32r recip is fine"):
                        nc.vector.reciprocal(recip_t[:],
                                             rs[32 * hi:32 * hi + 1, :])
                    bc = spanB([HD, 512], name="bc")
                    nc.tensor.matmul(
                        bc[:], ones1x64[:], recip_t[:], start=True, stop=True)
                    nc.vector.tensor_tensor(
                        wvt[hp:hp + HD, ec, qb * QB:(qb + 1) * QB],
                        wvt[hp:hp + HD, ec, qb * QB:(qb + 1) * QB],
                        bc[:], mybir.AluOpType.mult)

            if "ct" in mode:
                for qb in range(NQB):
                    quad = []
                    for h in range(H):
                        hp = (h % 2) * HD
                        ec = h // 2
                        e_t = epool.tile([P, NSC, 512], bf16, tag="E",
                                         name="e_t")
                        for g0, kind in kc_groups:
                            gn = 4 if kind == "A" else 2
                            sp = spanA() if kind == "A" else spanB()
                            for i in range(gn):
                                kc = g0 + i
                                nc.tensor.matmul(
                                    sp[:, i, :],
                                    khT[hp:hp + HD, ec, kc * P:(kc + 1) * P],
                                    khT[hp:hp + HD, ec, qb * QB:(qb + 1) * QB],
                                    start=True, stop=True)
                            nc.scalar.activation(
                                e_t[:, g0:g0 + gn, :], sp[:, 0:gn, :],
                                mybir.ActivationFunctionType.Exp, scale=SCALE)
                        quad.append(e_t)
                        if h % 2 == 1:
                            emit_pv_pair(h // 2, qb, quad[-2], quad[-1])
                        if h % 4 == 3:
                            emit_rs_quad(h // 4, qb, quad)
                            quad = []
                    for qc0 in range(qb * 4, qb * 4 + 4, 2):
                        emit_outproj(qc0)
            else:
                pending = None
                backlog = []
                for qb in range(NQB if "phase0" not in mode else 0):
                    for h in range(H):
                        hp = (h % 2) * HD
                        ec = h // 2
                        e_t = epool.tile([P, NSC, 512], bf16, tag="E", name="e_t")
                        for g0, kind in kc_groups:
                            gn = 4 if kind == "A" else 2
                            sp = spanA() if kind == "A" else spanB()
                            for i in range(gn):
                                kc = g0 + i
                                nc.tensor.matmul(
                                    sp[:, i, :],
                                    khT[hp:hp + HD, ec, kc * P:(kc + 1) * P],
                                    khT[hp:hp + HD, ec, qb * QB:(qb + 1) * QB],
                                    start=True, stop=True)
                            nc.scalar.activation(
                                e_t[:, g0:g0 + gn, :], sp[:, 0:gn, :],
                                mybir.ActivationFunctionType.Exp, scale=SCALE)
                        if pending is not None and "full" in mode:
                            emit_pv(*pending)
                            if pending[0] == H - 1:
                                backlog.extend(
                                    range(pending[1] * 4,
                                          pending[1] * 4 + 4, 2))
                            if (backlog and "spread" in mode
                                    and pending[0] % 2 == 1):
                                emit_outproj(backlog.pop(0))
                            elif backlog and "spread" not in mode:
                                while backlog:
                                    emit_outproj(backlog.pop(0))
                        pending = (h, qb, e_t)
                if "full" in mode:
                    emit_pv(*pending)
                    backlog.extend(
                        range(pending[1] * 4, pending[1] * 4 + 4, 2))
                    for qc0 in backlog:
                        emit_outproj(qc0)

    nc.compile()
    return nc


def _get_nc(repeat: int = 1, mode: str = "full"):
    key = ("nc", repeat, mode)
    if key not in _CACHE:
        _CACHE[key] = _build_nc(repeat, mode)
    return _CACHE[key]


def _shard_inputs(x, Wk, Wo):
    in_maps = []
    for c in range(NCORES):
        b, half = c // 2, c % 2
        xb = x[b]
        if half:
            xb = np.roll(xb, -SH, axis=0)
        in_maps.append({"x": np.ascontiguousarray(xb), "Wk": Wk, "Wo": Wo})
    return in_maps


def kernel(x: np.ndarray, Wk: np.ndarray, Wo: np.ndarray, _trace=False):
    from concourse import bass_utils

    nc = _get_nc()
    x = np.asarray(x, dtype=np.float32)
    Wk = np.ascontiguousarray(np.asarray(Wk, dtype=np.float32))
    Wo = np.ascontiguousarray(np.asarray(Wo, dtype=np.float32))

    in_maps = _shard_inputs(x, Wk, Wo)

    res = bass_utils.run_bass_kernel_spmd(
        nc, in_maps, core_ids=list(range(NCORES)), trace=_trace)

    out = np.empty((B, S, D), dtype=np.float32)
    for c in range(NCORES):
        b, half = c // 2, c % 2
        out[b, half * SH:(half + 1) * SH] = res.results[c]["out"]
    if _trace:
        _CACHE["last_results"] = res
    return out



# revision 25
# speedup vs baseline: 1.3961x; 1.3961x over previous
"""Trainium2 Bass kernel for MultiHeadSelfAttention (K-only variant), v2.

Math (per batch b):
    K  = x @ Wk.T;  Kh = heads(K)
    S_h = Kh @ Kh.T / sqrt(D);  P_h = softmax(S_h)
    wV_h = P_h @ Kh  (V == K);  out = concat_h(wV) @ Wo.T

Sharding (8 cores): core c handles batch c//2 and query-half c%2 with all
heads.  The query half is selected by rolling x on the host so each core
always computes queries 0:S//2 of its (rolled) sequence; softmax over keys
is order-invariant so rolling the key axis is harmless.

Per-core pipeline (one SPMD NEFF):
    xT    = PE-transpose(x)  fp32 -> bf16 on psum->sbuf copy    [d, s]
    khT   = WkT.T @ xT   (bf16 matmuls, directly in K^T layout) [e, s]
    k_bf  = XBAR-DMA-transpose(khT)                             [s, e]
    per (qb, head-pair j):  (heads 2j, 2j+1 live in partitions 0:64 /
                             64:128 of khT chunk ec=j)
      scores strip S_T[k, q] for both heads concurrently via PE row
        tiling (K=64 each, tile_position (0,0)/(64,0))
      exp: split between ScalarE (true Exp) and VectorE (Schraudolph
        bit-trick: int16 = round(s*A + B), bitcast to bf16)
      PV: col-tiled pair (M=64 at cols 0/64) accumulating over kc
      rowsums: M=1 matmuls col-tiled at 0/32 accumulating over kc
      recip (DVE) -> partition-broadcast (GpSimd) -> normalize (DVE)
    out = wvt.T @ WoT  (bf16), overlapped with the next query block
"""

import sys

if "/opt/trn_rl_repo" not in sys.path:
    sys.path.insert(0, "/opt/trn_rl_repo")

import numpy as np

B, S, D = 4, 2048, 512
H = 8
HD = D // H            # 64
P = 128
SH = S // 2            # query half per core = 1024
NCORES = 8
SCALE = 1.0 / np.sqrt(D)
SCHR_A = float(128.0 * SCALE * np.log2(np.e))
SCHR_B = 16250.0

_CACHE = {}


def _build_nc(repeat: int = 1, mode: str = "v2"):
    import concourse.bass as bass  # noqa: F401
    import concourse.tile as tile
    import concourse.mybir as mybir
    from concourse import bacc
    from concourse.masks import make_identity
    from contextlib import ExitStack
    import contextlib

    f32 = mybir.dt.float32
    bf16 = mybir.dt.bfloat16
    i16 = mybir.dt.int16

    nc = bacc.Bacc("TRN2", target_bir_lowering=False, debug=False,
                   num_devices=NCORES)

    x_d = nc.dram_tensor("x", [S, D], f32, kind="ExternalInput").ap()
    wk_d = nc.dram_tensor("Wk", [D, D], f32, kind="ExternalInput").ap()
    wo_d = nc.dram_tensor("Wo", [D, D], f32, kind="ExternalInput").ap()
    out_d = nc.dram_tensor("out", [SH, D], f32, kind="ExternalOutput").ap()

    NSC = S // P           # 16 sequence chunks
    NDC = D // P           # 4 feature chunks
    NQB = SH // 512        # 2 query blocks of 512
    QB = 512

    # which kc-groups go to the DVE (Schraudolph) vs ScalarE (Exp)
    n_dve = 6
    for tok in mode.split("_"):
        if tok.startswith("s") and tok[1:].isdigit():
            n_dve = int(tok[1:])
    dve_kcs = {2, 5, 8, 11, 13, 15, 1, 4, 7, 10}
    dve_kcs = set(sorted(dve_kcs)[:n_dve]) if n_dve <= 10 else set(
        range(16))

    Exp = mybir.ActivationFunctionType.Exp
    mult = mybir.AluOpType.mult
    add = mybir.AluOpType.add

    with tile.TileContext(nc) as tc:
        loop_cm = tc.For_i(0, repeat, 1) if repeat > 1 else (
            contextlib.nullcontext())
        with loop_cm, ExitStack() as ctx:
            consts = ctx.enter_context(tc.tile_pool(name="consts", bufs=1))
            kpool = ctx.enter_context(tc.tile_pool(name="kpool", bufs=1))
            epool = ctx.enter_context(tc.tile_pool(name="epool", bufs=1))
            vpool = ctx.enter_context(tc.tile_pool(name="vpool", bufs=1))
            opool = ctx.enter_context(tc.tile_pool(name="opool", bufs=1))
            ps = ctx.enter_context(
                tc.tile_pool(name="ps", bufs=1, space="PSUM"))

            ident = consts.tile([P, P], f32)
            make_identity(nc, ident[:])
            ones_bf = consts.tile([P, 1], bf16)
            nc.gpsimd.memset(ones_bf[:], 1.0)
            ones_r = consts.tile([1, HD], bf16)
            nc.gpsimd.memset(ones_r[:], 1.0)

            wkT = consts.tile([P, NDC, D], bf16)     # [d, dc? -> d-part, dc, e]
            woT = consts.tile([P, NDC, D], bf16)     # [e-part, ec, d']
            khT = kpool.tile([P, NDC, S], bf16)      # K^T [e, s]
            k_bf = kpool.tile([P, NSC, D], bf16)     # K [s, e]
            wvt = vpool.tile([P, NDC, SH], bf16)     # wV^T (normed in place)

            # table-set preload for Exp (overlaps the front-end DMAs)
            warm = consts.tile([1, 2], bf16)
            warmsrc = consts.tile([1, 2], f32)
            nc.gpsimd.memset(warmsrc[:], 0.0)
            nc.scalar.activation(warm[:], warmsrc[:], Exp, scale=SCALE)

            # ---- phase 0: weights + x transposes + khT + k_bf ----------
            # psum spans reuse the steady-state "sc" tag (2 banks each)
            with tc.tile_pool(name="stage", bufs=1) as stage:
                wk_sb = stage.tile([P, NDC, D], f32, tag="w", name="wk_sb")
                nc.sync.dma_start(
                    wk_sb[:], wk_d.rearrange("(eo p) d -> p eo d", p=P))
                for dch in range(2):
                    sp = ps.tile([P, 2, 512], f32, tag="sc", bufs=2,
                                 name="wkt_sp")
                    for i in range(2):
                        dc = dch * 2 + i
                        for eo in range(NDC):
                            nc.tensor.transpose(
                                sp[:, i, eo * P:(eo + 1) * P],
                                wk_sb[:, eo, dc * P:(dc + 1) * P],
                                ident[:])
                    nc.vector.tensor_copy(
                        wkT[:, dch * 2:dch * 2 + 2, :], sp[:])

                xT = stage.tile([P, NDC, S], bf16, tag="xT", name="xT")
                for g in range(8):
                    g0 = g * 2
                    x_g = stage.tile([P, 2, 512], f32, tag="x", bufs=3,
                                     name="x_g")
                    nc.sync.dma_start(
                        x_g[:, 0:2, :],
                        x_d[g0 * P:(g0 + 2) * P, :].rearrange(
                            "(two p) d -> p two d", p=P))
                    spt = ps.tile([P, 2, 512], f32, tag="sc", bufs=2,
                                  name="xt_sp")
                    for i in range(2):
                        for dc in range(NDC):
                            nc.tensor.transpose(
                                spt[:, i, dc * P:(dc + 1) * P],
                                x_g[:, i, dc * P:(dc + 1) * P], ident[:])
                    # spt free layout (i, dc, c); xT wants (dc, i, c)
                    src = spt[:].rearrange("p i (dc c) -> p dc i c", dc=4)
                    if g % 2 == 0:
                        nc.vector.tensor_copy(
                            xT[:, :, g0 * P:(g0 + 2) * P].rearrange(
                                "p dc (i c) -> p dc i c", i=2), src)
                    else:
                        nc.scalar.copy(
                            xT[:, :, g0 * P:(g0 + 2) * P].rearrange(
                                "p dc (i c) -> p dc i c", i=2), src)

                wo_sb = stage.tile([P, NDC, D], f32, tag="w2", name="wo_sb")
                nc.sync.dma_start(
                    wo_sb[:], wo_d.rearrange("(do p) e -> p do e", p=P))

                for sb in range(4):
                    for ech in range(2):
                        spk = ps.tile([P, 2, 512], f32, tag="sc", bufs=2,
                                      name="kt_sp")
                        for i in range(2):
                            ec = ech * 2 + i
                            for dc in range(NDC):
                                nc.tensor.matmul(
                                    spk[:, i, :],
                                    wkT[:, dc, ec * P:(ec + 1) * P],
                                    xT[:, dc, sb * 512:(sb + 1) * 512],
                                    start=(dc == 0), stop=(dc == NDC - 1))
                        dst = khT[:, ech * 2:ech * 2 + 2,
                                  sb * 512:(sb + 1) * 512]
                        if ech == 0:
                            nc.vector.tensor_copy(dst, spk[:])
                        else:
                            nc.scalar.copy(dst, spk[:])
                    for ec in range(NDC):
                        nc.sync.dma_start_transpose(
                            k_bf[:, 4 * sb:4 * sb + 4,
                                 ec * P:(ec + 1) * P],
                            khT[:, ec, sb * 512:(sb + 1) * 512])

                # Wo transpose last (PE) -> woT
                for ech in range(2):
                    spw = ps.tile([P, 2, 512], f32, tag="sc", bufs=2,
                                  name="wot_sp")
                    for i in range(2):
                        ec = ech * 2 + i
                        for do in range(NDC):
                            nc.tensor.transpose(
                                spw[:, i, do * P:(do + 1) * P],
                                wo_sb[:, do, ec * P:(ec + 1) * P],
                                ident[:])
                    nc.vector.tensor_copy(
                        woT[:, ech * 2:ech * 2 + 2, :], spw[:])

            # ---- steady state: (qb, head-pair) periods -----------------
            def emit_scores(j, qb, e_t, hooks):
                for kc in range(NSC):
                    sp = ps.tile([P, 2, 512], f32, tag="sc", bufs=2,
                                 name="sc")
                    for hi in range(2):
                        nc.tensor.matmul(
                            sp[:, hi, :],
                            khT[hi * HD:(hi + 1) * HD, j,
                                kc * P:(kc + 1) * P],
                            khT[hi * HD:(hi + 1) * HD, j,
                                qb * QB:(qb + 1) * QB],
                            start=True, stop=True,
                            tile_position=(hi * HD, 0))
                    dst = e_t[:, kc, :, :]
                    if kc in dve_kcs:
                        with nc.allow_low_precision(
                                reason="schraudolph exp"):
                            nc.vector.tensor_scalar(
                                dst.bitcast(i16), sp[:],
                                SCHR_A, SCHR_B, mult, add)
                    else:
                        nc.scalar.activation(dst, sp[:], Exp, scale=SCALE)
                    if kc in hooks:
                        hooks[kc]()

            def emit_pv(j, qb, e_t):
                # each head's PV chain gets its own bank and its own
                # proper accumulation group; the pair still runs
                # concurrently via col tiling (pv_b outputs at base 64
                # of its own bank so walrus' base==col rule holds)
                pv_a = ps.tile([P, 512], f32, tag="pv", bufs=2,
                               name="pv_a")
                pv_b = ps.tile([P, 512], f32, tag="pv", bufs=2,
                               name="pv_b")
                for kc in range(NSC):
                    nc.tensor.matmul(
                        pv_a[0:HD, :],
                        k_bf[:, kc, (2 * j) * HD:(2 * j + 1) * HD],
                        e_t[:, kc, 0, :],
                        start=(kc == 0), stop=(kc == NSC - 1),
                        tile_position=(0, 0))
                    nc.tensor.matmul(
                        pv_b[HD:P, :],
                        k_bf[:, kc, (2 * j + 1) * HD:(2 * j + 2) * HD],
                        e_t[:, kc, 1, :],
                        start=(kc == 0), stop=(kc == NSC - 1),
                        tile_position=(0, HD))
                # rowsums: separate bank per head, both chains at
                # partition 0 so the downstream recip/broadcast operands
                # all live at partition 0 (serialized on PE: same tile
                # position)
                rs_a = ps.tile([1, 512], f32, tag="rs", bufs=2,
                               name="rs_a")
                rs_b = ps.tile([1, 512], f32, tag="rs", bufs=2,
                               name="rs_b")
                for kc in range(NSC):
                    nc.tensor.matmul(
                        rs_a[0:1, :], ones_bf[:, 0:1], e_t[:, kc, 0, :],
                        start=(kc == 0), stop=(kc == NSC - 1))
                for kc in range(NSC):
                    nc.tensor.matmul(
                        rs_b[0:1, :], ones_bf[:, 0:1], e_t[:, kc, 1, :],
                        start=(kc == 0), stop=(kc == NSC - 1))
                return (pv_a, pv_b), (rs_a, rs_b)

            def emit_norm(j, qb, pv, rs):
                pv_a, pv_b = pv
                rs_a, rs_b = rs
                recip = vpool.tile([1, 2, 512], bf16, tag="recip", bufs=2,
                                   name="recip")
                with nc.allow_low_precision(reason="bf16 recip"):
                    nc.vector.reciprocal(recip[0:1, 0, :], rs_a[0:1, :])
                    nc.vector.reciprocal(recip[0:1, 1, :], rs_b[0:1, :])
                if dbg and j == 0 and qb == 0:
                    spvrs = vpool.tile([P, 3, 512], f32, tag="dpv",
                                       name="spvrs")
                    nc.vector.tensor_copy(spvrs[0:HD, 0, :],
                                          pv_a[0:HD, :])
                    nc.vector.tensor_copy(spvrs[HD:P, 0, :],
                                          pv_b[HD:P, :])
                    nc.gpsimd.memset(spvrs[:, 1:3, :], 0.0)
                    nc.vector.tensor_copy(spvrs[0:1, 1, :], rs_a[0:1, :])
                    nc.vector.tensor_copy(spvrs[0:1, 2, :], rs_b[0:1, :])
                    nc.sync.dma_start(d_pvrs, spvrs[:])
                # partition-broadcast each head's recip row via K=1
                # matmuls; all operands at partition 0, outputs col-tiled
                # to bases 0/64 (third alloc on the rs tag reuses rs_a's
                # bank once the recips have read it)
                rb = ps.tile([P, 512], f32, tag="rs", bufs=2, name="rb")
                nc.tensor.matmul(rb[0:HD, :], ones_r[0:1, :],
                                 recip[0:1, 0, :],
                                 start=True, stop=True,
                                 tile_position=(0, 0))
                nc.tensor.matmul(rb[HD:P, :], ones_r[0:1, :],
                                 recip[0:1, 1, :],
                                 start=True, stop=True,
                                 tile_position=(0, HD))
                if dbg and j == 0 and qb == 0:
                    srcp = vpool.tile([P, 3, 512], bf16, tag="drc",
                                      name="srcp")
                    nc.vector.tensor_copy(srcp[:, 0, :], rb[:])
                    nc.gpsimd.memset(srcp[:, 1:3, :], 0.0)
                    nc.vector.tensor_copy(srcp[0:1, 1, :],
                                          recip[0:1, 0, :])
                    nc.vector.tensor_copy(srcp[0:1, 2, :],
                                          recip[0:1, 1, :])
                    nc.sync.dma_start(d_rcp, srcp[:])
                nc.vector.tensor_copy(
                    wvt[0:HD, j, qb * QB:(qb + 1) * QB], pv_a[0:HD, :])
                nc.vector.tensor_copy(
                    wvt[HD:P, j, qb * QB:(qb + 1) * QB], pv_b[HD:P, :])
                nc.vector.tensor_tensor(
                    wvt[:, j, qb * QB:(qb + 1) * QB],
                    wvt[:, j, qb * QB:(qb + 1) * QB],
                    rb[:], mult)

            def emit_outproj(qb, half):
                # half in {0,1}: query chunks qb*4 + 2*half + {0,1}
                qc0 = qb * 4 + 2 * half
                po = ps.tile([P, 2, 512], f32, tag="sc", bufs=2, name="po")
                for j2 in range(2):
                    qc = qc0 + j2
                    for ec in range(NDC):
                        nc.tensor.matmul(
                            po[:, j2, :],
                            wvt[:, ec, qc * P:(qc + 1) * P],
                            woT[:, ec, :],
                            start=(ec == 0), stop=(ec == NDC - 1))
                o_sb = opool.tile([P, 2, 512], f32, tag="osb", bufs=2,
                                  name="o_sb")
                nc.vector.tensor_copy(o_sb[:], po[:])
                nc.sync.dma_start(
                    out_d[qc0 * P:(qc0 + 2) * P, :].rearrange(
                        "(two p) d -> p two d", p=P),
                    o_sb[:])

            dbg = "dbg" in mode
            if dbg:
                dpool = ctx.enter_context(
                    tc.tile_pool(name="dbg", bufs=1, space="DRAM"))
                d_kht = nc.dram_tensor(
                    "d_kht", [P, NDC, S], bf16, kind="ExternalOutput").ap()
                d_kbf = nc.dram_tensor(
                    "d_kbf", [P, NSC, D], bf16, kind="ExternalOutput").ap()
                d_et = nc.dram_tensor(
                    "d_et", [P, NSC, 2, 512], bf16,
                    kind="ExternalOutput").ap()
                d_pvrs = nc.dram_tensor(
                    "d_pvrs", [P, 3, 512], f32, kind="ExternalOutput").ap()
                d_rcp = nc.dram_tensor(
                    "d_rcp", [P, 3, 512], bf16, kind="ExternalOutput").ap()
                d_wvt = nc.dram_tensor(
                    "d_wvt", [P, NDC, SH], bf16, kind="ExternalOutput").ap()
                nc.sync.dma_start(d_kht, khT[:])
                nc.sync.dma_start(d_kbf, k_bf[:])

            pending = None
            outq = []
            for qb in range(NQB):
                for j in range(4):
                    e_t = epool.tile([P, NSC, 2, 512], bf16, tag="E",
                                     bufs=2, name="e_t")
                    hooks = {}
                    if pending is not None:
                        pj, pqb, pe = pending
                        state = {}

                        def h3(pj=pj, pqb=pqb, pe=pe, state=state):
                            state["pvrs"] = emit_pv(pj, pqb, pe)

                        def h8(pj=pj, pqb=pqb, state=state):
                            emit_norm(pj, pqb, *state["pvrs"])
                            if pj == 3:
                                outq.extend([(pqb, 0), (pqb, 1)])

                        def h11():
                            if outq:
                                emit_outproj(*outq.pop(0))

                        def h13():
                            if outq:
                                emit_outproj(*outq.pop(0))

                        hooks = {3: h3, 8: h8, 11: h11, 13: h13}
                    emit_scores(j, qb, e_t, hooks)
                    if dbg and j == 0 and qb == 0:
                        nc.sync.dma_start(d_et, e_t[:])
                    pending = (j, qb, e_t)

            pj, pqb, pe = pending
            pv, rs = emit_pv(pj, pqb, pe)
            emit_norm(pj, pqb, pv, rs)
            outq.extend([(pqb, 0), (pqb, 1)])
            while outq:
                emit_outproj(*outq.pop(0))
            if dbg:
                nc.sync.dma_start(d_wvt, wvt[:])

    nc.compile()
    return nc


def _get_nc(repeat: int = 1, mode: str = "v2"):
    key = ("nc", repeat, mode)
    if key not in _CACHE:
        _CACHE[key] = _build_nc(repeat, mode)
    return _CACHE[key]


def _shard_inputs(x, Wk, Wo):
    in_maps = []
    for c in range(NCORES):
        b, half = c // 2, c % 2
        xb = x[b]
        if half:
            xb = np.roll(xb, -SH, axis=0)
        in_maps.append({"x": np.ascontiguousarray(xb), "Wk": Wk, "Wo": Wo})
    return in_maps


def kernel(x: np.ndarray, Wk: np.ndarray, Wo: np.ndarray, _trace=False):
    from concourse import bass_utils

    nc = _get_nc()
    x = np.asarray(x, dtype=np.float32)
    Wk = np.ascontiguousarray(np.asarray(Wk, dtype=np.float32))
    Wo = np.ascontiguousarray(np.asarray(Wo, dtype=np.float32))

    in_maps = _shard_inputs(x, Wk, Wo)

    res = bass_utils.run_bass_kernel_spmd(
        nc, in_maps, core_ids=list(range(NCORES)), trace=_trace)

    out = np.empty((B, S, D), dtype=np.float32)
    for c in range(NCORES):
        b, half = c // 2, c % 2
        out[b, half * SH:(half + 1) * SH] = res.results[c]["out"]
    if _trace:
        _CACHE["last_results"] = res
    return out
